# revision 62
# baseline (speedup 1.0000x reference)
"""Causal self-attention (B=4, T=2048, C=1024, H=16, hs=64) on 8 trn2 cores.

Sharding: core c = batch (c//2) x head-group (c%2, 8 heads each).
Each core computes, for its (batch, 8 heads):
  - QKV projection against its slice of w_attn (transposed layouts on chip),
  - causal softmax attention (flash-style, no max subtraction -- scores are
    O(1) for this problem so exp is numerically safe; softmax denominator
    comes for free as a 65th "ones" row appended to V in the PV matmul),
  - partial output projection against its 512 rows of w_o, transposed,
    stored as bf16 partials.
Host side: per-batch pair partials are summed in fp32 (the tensor-parallel
all-reduce done at unshard time) and transposed back.

Performance notes (HW-measured on trn2, For_i-delta timing):
  - All matmuls are plain K=128/M=128 shapes: the S matmuls use per-head
    zero-padded K arrays and PV uses an M-padded V' — the K=64 row-tiled /
    M=65 variants measured ~+55ns/matmul (slow decode path).
  - The causal mask is injected by a [128,128] maskT @ identity matmul
    seeding the diagonal PSUM block (start=True); S accumulates onto it.
    No DVE mask-add sits on the S->exp chain.
  - exp is sliced to skip fully-masked query columns of diagonal tiles.
  - The softmax 1/l scale runs as per-half-qtile gather -> reciprocal ->
    DRAM-bounce broadcast; the last output chunk's contraction is split
    kc01/kc23 so its first half fills the PE gap during the final chain.
"""

import numpy as np
from ml_dtypes import bfloat16

import concourse.bass as bass
import concourse.mybir as mybir
import concourse.tile as tile

N_CORES = 8
B, T, C = 4, 2048, 1024
H_PER_CORE = 8
HS = 64
P = 128
KCH = C // P  # 8 contraction chunks for the projections
NCH = T // 512  # 4 token chunks of 512
QT = T // 512  # 4 query tiles of 512
NEG = -1.0e5
SCALE = 1.0 / np.sqrt(HS)

F32 = mybir.dt.float32
BF16 = mybir.dt.bfloat16


def _mm(nc, out, lhsT, rhs, **kw):
    """Matmul on bf16 operands (1 cycle/row; plain fp32 costs 4)."""
    nc.tensor.matmul(out, lhsT, rhs, **kw)


def legalize_sync_waits(nc, max_waits=1):
    """Split multi-sem-wait instructions into chains of single-wait nops.

    The installed walrus codegen rejects instructions whose sync_info
    carries more than one wait. Same-engine program order makes moving the
    extra waits onto preceding nops semantically identical.
    """
    for f in nc.m.functions:
        for b in f.blocks:
            new_insts = []
            for inst in b.instructions:
                si = inst.sync_info
                if si and si.on_wait and len(si.on_wait) > max_waits:
                    waits = list(si.on_wait)
                    extra, keep = waits[:-max_waits], waits[-max_waits:]
                    for i in range(0, len(extra), max_waits):
                        chunk = extra[i : i + max_waits]
                        nop = mybir.InstNoOp(
                            name=f"{inst.name}-ws{i}",
                            engine=inst.engine,
                            ins=[],
                            outs=[],
                            sync_info=mybir.SyncInfo(on_wait=chunk, on_update=[]),
                        )
                        new_insts.append(nop)
                    inst.sync_info = mybir.SyncInfo(
                        on_wait=keep, on_update=list(si.on_update or [])
                    )
                new_insts.append(inst)
            b.instructions = new_insts


def build_attention_body(nc, tc, ctx, tensors, opts=None):
    """Emit one iteration of the per-core attention computation.

    Emission order interleaves the three phases (QKV projection chunks,
    attention qtiles, output-projection chunks) so every engine sees work
    early: engines execute their instruction streams in program order, so
    emitting all of phase 1 first would serialize ACT behind PE.
    """
    x_t, w_qk, w_v, w_o, m_t, ident, out_t = tensors
    opts = dict(opts or {})
    phases = opts.get("phases", (1, 2, 3))
    Exp = mybir.ActivationFunctionType.Exp

    x_t3 = x_t[:].rearrange("(ko ki) t -> ki ko t", ki=P)  # [128, 8, 2048]
    w_qk3 = w_qk[:].rearrange("(ko ki) m -> ki ko m", ki=P)  # [128, 8, 1024]
    w_v3 = w_v[:].rearrange("(ko ki) m -> ki ko m", ki=P)  # [128, 8, 512]
    w_o3 = w_o[:].rearrange("(ko ki) m -> ki ko m", ki=P)  # [128, 4, 1024]
    out2 = out_t[:]  # [1024, 2048]

    consts = ctx.enter_context(tc.tile_pool(name="consts", bufs=1))
    big = ctx.enter_context(tc.tile_pool(name="big", bufs=1))
    expp = ctx.enter_context(tc.tile_pool(name="expp", bufs=opts.get("expp_bufs", 4)))
    smallp = ctx.enter_context(tc.tile_pool(name="smallp", bufs=2))
    dramp = ctx.enter_context(tc.tile_pool(name="dramp", bufs=2, space="DRAM"))
    outsb = ctx.enter_context(tc.tile_pool(name="outsb", bufs=2))
    psum = ctx.enter_context(tc.tile_pool(name="psum", bufs=2, space="PSUM"))

    # Resident inputs and per head-pair Q^T / K^T / Y^T tiles.
    s_kpad = opts.get("s_kpad", True)
    pv_pad = opts.get("pv_pad", True)
    VW = 2 * HS if pv_pad else HS + 1
    xt_sb = big.tile([P, KCH, T], BF16, name="xt_sb")
    wqk_sb = big.tile([P, KCH, 1024], BF16, name="wqk_sb")
    q_sb = [big.tile([P, T], BF16, name=f"q_sb{p}") for p in range(4)]
    # With s_kpad, each head's K^T lives in its own zero-padded [128, T]
    # array so the S matmul is a plain full-array K=128 matmul (the zero
    # rows annihilate the other head's q rows). Without it, the two heads
    # share one array at partition offsets 0/64 (row-tiled matmuls).
    if s_kpad:
        k_sb = [
            [big.tile([P, T], BF16, name=f"k_sb{p}_{i}") for i in (0, 1)]
            for p in range(4)
        ]
    else:
        k_sb = [big.tile([P, T], BF16, name=f"k_sb{p}") for p in range(4)]
    y_sb = [big.tile([P, T], BF16, name=f"y_sb{p}") for p in range(4)]
    # V' = [V | 1 | pad] per head: [key%128, keychunk, head, VW]; the pad
    # columns (pv_pad) make the PV matmul a full M=128 stationary so it
    # stays on the fast decode path — psum rows past 65 are never read.
    vp_sb = big.tile([P, T // P, H_PER_CORE, VW], BF16, name="vp_sb")

    # DMA emission order = scheduler priority: the first QK matmul needs
    # x chunk 0 + the first w_qk tile, so those go first; w_v before the
    # first V matmul; mask is tiny; w_o is deferred (a "w_o" step below)
    # since phase 3 starts ~100us in.
    # Causal masking is injected on the PE itself: a [128,128] matmul of
    # maskT against the identity seeds the diagonal PSUM block with NEG
    # (start=True), and the S matmul accumulates onto it (start=False).
    # This keeps the S->exp chain free of a cross-engine DVE mask-add.
    mt_sb = consts.tile([P, P], BF16, name="mt_sb")
    id_sb = consts.tile([P, P], BF16, name="id_sb")
    w_v_sb = consts.tile([P, KCH, 512], BF16, name="w_v_sb")
    w_o_sb = consts.tile([P, 4, 1024], BF16, name="w_o_sb")
    ones_sb = consts.tile([1, HS], BF16, name="ones_sb")
    nc.vector.memset(ones_sb, 1.0)
    nc.vector.memset(vp_sb[:, :, :, HS : HS + 1], 1.0)
    cew = None
    if opts.get("noexp"):
        # Probe mode: PV reads a constant tile instead of exp(S); output is
        # garbage but the PE/DVE pipeline shape is preserved minus ACT.
        cew = consts.tile([P, 2, 512], BF16, name="cew")
        nc.vector.memset(cew, 0.001)

    SW_BUFS = opts.get("sw_bufs", 2)
    O_BUFS = opts.get("o_bufs", 2)
    P1_BUFS = opts.get("p1_bufs", 2)
    evict_any = opts.get("evict", "vector") == "any"

    def evict(out, in_):
        # PSUM evictions stay off ACT (the exp bottleneck); gpsimd has no
        # PSUM port, so pin them to DVE unless opted back to any.
        if evict_any:
            nc.any.tensor_copy(out=out, in_=in_)
        else:
            nc.vector.tensor_copy(out=out, in_=in_)

    def phase1_chunk(nch):
        ts0 = nch * 512
        if nch == 0:
            # Fine-grained first loads so the very first matmul (mt0, kc0)
            # only waits for ~0.5MB: x kc0-1 + the first w_qk tile.
            # First x piece on the sync queue, first weight tiles on the
            # scalar queue: the two HWDGE queues overlap, so the first
            # matmul's inputs land after ~0.5MB each way.
            nc.sync.dma_start(
                out=xt_sb[:, 0:2, 0:512], in_=x_t3[:, 0:2, 0:512]
            )
            nc.scalar.dma_start(
                out=wqk_sb[:, :, 0:P], in_=w_qk3[:, :, 0:P]
            )
            nc.sync.dma_start(
                out=xt_sb[:, 2:4, 0:512], in_=x_t3[:, 2:4, 0:512]
            )
            nc.scalar.dma_start(
                out=wqk_sb[:, :, 4 * P : 5 * P], in_=w_qk3[:, :, 4 * P : 5 * P]
            )
            nc.sync.dma_start(
                out=xt_sb[:, 4:8, 0:512], in_=x_t3[:, 4:8, 0:512]
            )
            # Weight-tile DMA order matches the reordered unit order below
            # (Q0, K0, V first) so attn(0, p=0) can start ~10us earlier,
            # overlapping the first exps with the tail of the startup DMAs.
            nc.sync.dma_start(out=w_v_sb, in_=w_v3)
            nc.scalar.dma_start(out=mt_sb, in_=m_t[:])
            nc.scalar.dma_start(out=id_sb, in_=ident[:])
            for mt, eng in ((1, nc.scalar), (5, nc.sync), (2, nc.scalar),
                            (6, nc.sync), (3, nc.scalar), (7, nc.sync)):
                eng.dma_start(
                    out=wqk_sb[:, :, mt * P : (mt + 1) * P],
                    in_=w_qk3[:, :, mt * P : (mt + 1) * P],
                )
        else:
            for kh in (0, 4):
                nc.sync.dma_start(
                    out=xt_sb[:, kh : kh + 4, ts0 : ts0 + 512],
                    in_=x_t3[:, kh : kh + 4, ts0 : ts0 + 512],
                )
        def qk_unit(mt):
            ps = psum.tile(
                [P, 512], F32, name=f"p1_{nch}_{mt}", tag="p1", bufs=P1_BUFS
            )
            for kc in range(KCH):
                _mm(
                    nc,
                    ps,
                    wqk_sb[:, kc, mt * P : (mt + 1) * P],
                    xt_sb[:, kc, ts0 : ts0 + 512],
                    start=(kc == 0),
                    stop=(kc == KCH - 1),
                )
            if mt < 4:
                evict(q_sb[mt][:, ts0 : ts0 + 512], ps)
            elif s_kpad:
                kp = k_sb[mt - 4]
                evict(kp[0][0:HS, ts0 : ts0 + 512], ps[0:HS, :])
                evict(kp[1][HS:P, ts0 : ts0 + 512], ps[HS:P, :])
                nc.gpsimd.memset(kp[0][HS:P, ts0 : ts0 + 512], 0.0)
                nc.gpsimd.memset(kp[1][0:HS, ts0 : ts0 + 512], 0.0)
            else:
                evict(k_sb[mt - 4][:, ts0 : ts0 + 512], ps)

        # Produce Q pair 0 / K pair 0 / V first, then interleave this
        # chunk's attention pair-by-pair with the remaining QK units: the
        # exp stream for attn(nch) starts as soon as its first pair's
        # inputs exist instead of after the whole projection chunk.
        for mt in (0, 4):
            qk_unit(mt)
        for tt in range(4):  # V for token chunks of 128
            ps = psum.tile(
                [P, 512], F32, name=f"pv_{nch}_{tt}", tag="p1", bufs=P1_BUFS
            )
            for kc in range(KCH):
                _mm(
                    nc,
                    ps,
                    xt_sb[:, kc, ts0 + tt * P : ts0 + (tt + 1) * P],
                    w_v_sb[:, kc, :],
                    start=(kc == 0),
                    stop=(kc == KCH - 1),
                )
            kchunk = nch * 4 + tt
            evict(
                vp_sb[:, kchunk, :, 0:HS],
                ps.rearrange("p (h d) -> p h d", h=H_PER_CORE),
            )
        if 2 in phases:
            attn_qtile(nch, pairs=(0,))
        for p in (1, 2, 3):
            qk_unit(p)
            qk_unit(p + 4)
            if 2 in phases:
                attn_qtile(nch, pairs=(p,))

    osb_tiles = {}
    dq_tiles = {}

    def attn_tail_head(qt, h, o_ps):
        # Evict psum early (frees the o bank) including the denominator row;
        # gather that row into the per-qtile SBUF tile (SBUF->SBUF DMA) so
        # one reciprocal serves 4 heads at a time.
        o_sb = smallp.tile([HS + 1, 512], BF16, name=f"os_{qt}_{h}", tag="osb", bufs=8)
        nc.vector.tensor_copy(out=o_sb, in_=o_ps[0 : HS + 1, :])
        osb_tiles[(qt, h)] = o_sb
        half = h // 4
        if (qt, half) not in dq_tiles:
            dq_tiles[(qt, half)] = smallp.tile(
                [4, 512], BF16, name=f"dq_{qt}_{half}", tag="dsb", bufs=4
            )
        nc.sync.dma_start(
            out=dq_tiles[(qt, half)][h % 4 : h % 4 + 1, :],
            in_=o_sb[HS : HS + 1, :],
        )

    def attn_tail_half(qt, half):
        # One [4, 512] reciprocal per half-qtile (DVE reciprocal cost scales
        # with free size only), one broadcast DMA (DRAM bounce) for 4 heads,
        # 1/l multiplies on DVE. Halves are emitted as soon as their two
        # head-pairs finish so the chain overlaps remaining attention work.
        q0 = qt * 512
        h0 = half * 4
        recip = smallp.tile([4, 512], BF16, name=f"r_{qt}_{half}", tag="recip", bufs=4)
        with nc.allow_low_precision(reason="1/l in bf16 for the broadcast"):
            nc.vector.reciprocal(out=recip, in_=dq_tiles.pop((qt, half)))
        rd = dramp.tile([4, 512], BF16, name=f"rd_{qt}_{half}", tag="rd", bufs=4)
        nc.sync.dma_start(out=rd, in_=recip)
        bc_sb = smallp.tile([HS, 4, 512], BF16, name=f"bs_{qt}_{half}", tag="bcs", bufs=2)
        nc.sync.dma_start(out=bc_sb, in_=rd[:].partition_broadcast(HS))
        for hh in range(4):
            h = h0 + hh
            p, r0 = h // 2, (h % 2) * HS
            nc.any.tensor_mul(
                out=y_sb[p][r0 : r0 + HS, q0 : q0 + 512],
                in0=osb_tiles.pop((qt, h))[0:HS, :],
                in1=bc_sb[:, hh, :],
            )

    def attn_qtile(qt, pairs=(0, 1, 2, 3)):
        # Key chunks go two at a time into a 2-bank psum tile so one
        # FD=1024 exp covers both. Fully-masked column ranges are never
        # S-computed; the exp of whatever stale psum sits there is written
        # to sbuf but never read.
        q0 = qt * 512
        nkc = 4 * (qt + 1)
        for p in pairs:
            o_ab = [
                psum.tile(
                    [P if pv_pad else HS + 1, 512], F32,
                    name=f"o_{qt}_{p}_{i}", tag="o", bufs=O_BUFS,
                )
                for i in (0, 1)
            ]
            for kc2 in range(0, nkc, 2):
                sw_ab = [
                    psum.tile(
                        [P, 2, 512], F32, name=f"sw_{qt}_{p}_{kc2}_{i}",
                        tag="sw", bufs=SW_BUFS,
                    )
                    for i in (0, 1)
                ]
                for ci in (0, 1):
                    kc = kc2 + ci
                    c = kc - 4 * qt
                    qoff = max(0, c) * P
                    for i in (0, 1):
                        r0 = i * HS
                        if c >= 0:
                            _mm(
                                nc,
                                sw_ab[i][:, ci, qoff : qoff + P],
                                mt_sb,
                                id_sb,
                                start=True,
                                stop=False,
                            )
                        if s_kpad:
                            lhsT = k_sb[p][i][:, kc * P : (kc + 1) * P]
                            rhs = q_sb[p][:, q0 + qoff : q0 + 512]
                        else:
                            lhsT = k_sb[p][r0 : r0 + HS, kc * P : (kc + 1) * P]
                            rhs = q_sb[p][r0 : r0 + HS, q0 + qoff : q0 + 512]
                        _mm(
                            nc,
                            sw_ab[i][:, ci, qoff:],
                            lhsT,
                            rhs,
                            start=(c < 0),
                            stop=True,
                        )
                # Columns below qoff0 are never read by the PV matmuls
                # (fully masked), so don't spend ACT cycles on their exp.
                qoff0 = max(0, kc2 - 4 * qt) * P
                for i in (0, 1):
                    h = 2 * p + i
                    if cew is not None:
                        ew = cew
                    else:
                        ew = expp.tile(
                            [P, 2, 512], BF16, name=f"e_{qt}_{h}_{kc2}", tag="exps"
                        )
                        nc.scalar.activation(
                            out=ew[:, :, qoff0:],
                            in_=sw_ab[i][:, :, qoff0:],
                            func=Exp,
                            scale=SCALE,
                        )
                        if opts.get("exp2x"):
                            ew2 = expp.tile(
                                [P, 2, 512], BF16, name=f"e2_{qt}_{h}_{kc2}",
                                tag="exps2",
                            )
                            nc.scalar.activation(
                                out=ew2[:, :, qoff0:],
                                in_=sw_ab[i][:, :, qoff0:],
                                func=Exp,
                                scale=SCALE,
                            )
                    for ci in (0, 1):
                        kc = kc2 + ci
                        qoff = max(0, kc - 4 * qt) * P
                        _mm(
                            nc,
                            o_ab[i][:, qoff:],
                            vp_sb[:, kc, h, :],
                            ew[:, ci, qoff:],
                            start=(kc == 0),
                            stop=(kc == nkc - 1),
                        )
            for i in (0, 1):
                attn_tail_head(qt, 2 * p + i, o_ab[i])
            if p == 1:
                attn_tail_half(qt, 0)
            elif p == 3:
                attn_tail_half(qt, 1)

    def phase3_chunk(nch):
        ts0 = nch * 512
        for mt in range(8):
            if nch == 3:
                # Last chunk: y heads 4-7 arrive only after the final softmax
                # denominator chain. Split the contraction so the kc0/kc1
                # half (heads 0-3, ready early) fills the PE gap during that
                # chain; the halves are summed on eviction.
                psa = psum.tile(
                    [P, 512], F32, name=f"poa_{mt}", tag="p1", bufs=P1_BUFS
                )
                for kc in range(2):
                    _mm(
                        nc,
                        psa,
                        w_o_sb[:, kc, mt * P : (mt + 1) * P],
                        y_sb[kc][:, ts0 : ts0 + 512],
                        start=(kc == 0),
                        stop=(kc == 1),
                    )
                # Evict the early half right away so all 8 A-groups can run
                # during the final denominator chain (PSUM is only 8 banks).
                a_sb = outsb.tile([P, 512], BF16, name=f"a3_{mt}", tag="a3", bufs=8)
                evict(a_sb, psa)
                psb = psum.tile(
                    [P, 512], F32, name=f"pob_{mt}", tag="o", bufs=O_BUFS
                )
                for kc in range(2, 4):
                    _mm(
                        nc,
                        psb,
                        w_o_sb[:, kc, mt * P : (mt + 1) * P],
                        y_sb[kc][:, ts0 : ts0 + 512],
                        start=(kc == 2),
                        stop=(kc == 3),
                    )
                ot = outsb.tile([P, 512], BF16, name=f"ot_{nch}_{mt}", tag="ot", bufs=4)
                # ACT is idle by then, let the scheduler place the add.
                nc.any.tensor_add(out=ot, in0=a_sb, in1=psb)
            else:
                ps = psum.tile(
                    [P, 512], F32, name=f"po_{nch}_{mt}", tag="p1", bufs=P1_BUFS
                )
                for kc in range(4):
                    _mm(
                        nc,
                        ps,
                        w_o_sb[:, kc, mt * P : (mt + 1) * P],
                        y_sb[kc][:, ts0 : ts0 + 512],
                        start=(kc == 0),
                        stop=(kc == 3),
                    )
                ot = outsb.tile([P, 512], BF16, name=f"ot_{nch}_{mt}", tag="ot", bufs=4)
                evict(ot, ps)
            # Stores on the sync queue: loads are done by now, and issuing
            # descriptors from the ACT sequencer would delay exp dispatch.
            nc.sync.dma_start(
                out=out2[mt * P : (mt + 1) * P, ts0 : ts0 + 512], in_=ot
            )

    # Interleaved schedule; each step only uses data emitted before it.
    steps = []
    if 1 in phases:
        steps.append(("p1", 0))  # each p1 step includes its attn, interleaved
    if 1 in phases:
        steps.append(("p1", 1))
    steps.append(("w_o", 0))
    if 1 in phases:
        steps.append(("p1", 2))
    if 1 in phases:
        steps.append(("p1", 3))
    if 3 in phases:
        steps.append(("p3", 0))
        steps.append(("p3", 1))
        steps.append(("p3", 2))
        steps.append(("p3", 3))
    for kind, idx in steps:
        if kind == "p1":
            phase1_chunk(idx)
        elif kind == "attn":
            attn_qtile(idx)
        elif kind == "w_o":
            nc.sync.dma_start(out=w_o_sb, in_=w_o3)
        else:
            phase3_chunk(idx)


def build_nc(loop_k=None, opts=None):
    """Build the per-core Bass module. loop_k wraps the body in a timing loop."""
    from contextlib import ExitStack

    nc = bass.Bass("TRN2")
    x_t = nc.dram_tensor("x_t", [C, T], BF16, kind="ExternalInput")
    w_qk = nc.dram_tensor("w_qk", [C, 1024], BF16, kind="ExternalInput")
    w_v = nc.dram_tensor("w_v", [C, 512], BF16, kind="ExternalInput")
    w_o = nc.dram_tensor("w_o", [512, C], BF16, kind="ExternalInput")
    m_t = nc.dram_tensor("m_t", [P, P], BF16, kind="ExternalInput")
    ident = nc.dram_tensor("ident", [P, P], BF16, kind="ExternalInput")
    # bf16 partials: each core's contribution is one of two summands; the
    # fp32 sum happens host-side in unshard_output. Halves the store traffic.
    out_t = nc.dram_tensor("out_t", [C, T], BF16, kind="ExternalOutput")
    tensors = (x_t, w_qk, w_v, w_o, m_t, ident, out_t)

    dup = (opts or {}).get("dup", 1)
    with tile.TileContext(nc) as tc:
        with ExitStack() as ctx:
            if loop_k is None:
                for _ in range(dup):
                    build_attention_body(nc, tc, ctx, tensors, opts)
            else:
                with tc.For_i(0, loop_k, 1):
                    for _ in range(dup):
                        build_attention_body(nc, tc, ctx, tensors, opts)
    legalize_sync_waits(nc, max_waits=(opts or {}).get("max_waits", 1))
    return nc


def shard_inputs(x, w_attn, w_o):
    """Build the 8 per-core input maps."""
    x = np.asarray(x, dtype=np.float32)
    w_attn = np.asarray(w_attn, dtype=np.float32)
    w_o = np.asarray(w_o, dtype=np.float32)
    w_q, w_k, w_v = w_attn[:, 0:C], w_attn[:, C : 2 * C], w_attn[:, 2 * C : 3 * C]
    r = np.arange(P)
    mask = np.where(r[:, None] > r[None, :], np.float32(NEG), np.float32(0.0))
    m_t = np.ascontiguousarray(mask.T).astype(bfloat16)
    ident = np.eye(P, dtype=np.float32).astype(bfloat16)
    in_maps = []
    for c in range(N_CORES):
        b, g = c // 2, c % 2
        heads = range(g * H_PER_CORE, (g + 1) * H_PER_CORE)
        w_qk_c = np.concatenate(
            [w_q[:, h * HS : (h + 1) * HS] for h in heads]
            + [w_k[:, h * HS : (h + 1) * HS] for h in heads],
            axis=1,
        )
        w_v_c = np.concatenate([w_v[:, h * HS : (h + 1) * HS] for h in heads], axis=1)
        in_maps.append(
            {
                "x_t": np.ascontiguousarray(x[b].T).astype(bfloat16),
                "w_qk": np.ascontiguousarray(w_qk_c).astype(bfloat16),
                "w_v": np.ascontiguousarray(w_v_c).astype(bfloat16),
                "w_o": np.ascontiguousarray(
                    w_o[g * 512 : (g + 1) * 512, :]
                ).astype(bfloat16),
                "m_t": m_t,
                "ident": ident,
            }
        )
    return in_maps


def unshard_output(results):
    """Sum per-batch pair partials (the TP all-reduce) and untranspose."""
    out = np.empty((B, T, C), dtype=np.float32)
    for b in range(B):
        acc = results[2 * b]["out_t"].astype(np.float32) + results[2 * b + 1][
            "out_t"
        ].astype(np.float32)
        out[b] = acc.T
    return out


# ---------------------------------------------------------------------------
# PJRT SPMD execution (axon): jit a shard_map over the 8 cores.
# ---------------------------------------------------------------------------


class SpmdRunner:
    def __init__(self, nc, n_cores=N_CORES):
        import jax
        from jax.sharding import Mesh, PartitionSpec
        from jax.experimental.shard_map import shard_map
        from concourse.bass2jax import (
            _bass_exec_p,
            install_neuronx_cc_hook,
            partition_id_tensor,
        )

        install_neuronx_cc_hook()
        self.jax = jax
        self.n_cores = n_cores
        partition_name = nc.partition_id_tensor.name if nc.partition_id_tensor else None
        in_names, out_names, out_avals, zero_outs = [], [], [], []
        for alloc in nc.m.functions[0].allocations:
            if not isinstance(alloc, mybir.MemoryLocationSet):
                continue
            name = alloc.memorylocations[0].name
            if alloc.kind == "ExternalInput":
                if name != partition_name:
                    in_names.append(name)
            elif alloc.kind == "ExternalOutput":
                out_names.append(name)
                shape = tuple(alloc.tensor_shape)
                dtype = mybir.dt.np(alloc.dtype)
                out_avals.append(jax.core.ShapedArray(shape, dtype))
                zero_outs.append(np.zeros(shape, dtype))
        self.in_names, self.out_names = in_names, out_names
        self.out_avals, self.zero_outs = out_avals, zero_outs
        n_params, n_outs = len(in_names), len(out_avals)
        all_in_names = in_names + out_names
        if partition_name is not None:
            all_in_names.append(partition_name)

        def _body(*args):
            operands = list(args)
            if partition_name is not None:
                operands.append(partition_id_tensor())
            return tuple(
                _bass_exec_p.bind(
                    *operands,
                    out_avals=tuple(out_avals),
                    in_names=tuple(all_in_names),
                    out_names=tuple(out_names),
                    lowering_input_output_aliases=(),
                    sim_require_finite=True,
                    sim_require_nnan=True,
                    nc=nc,
                )
            )

        devices = jax.devices()[:n_cores]
        assert len(devices) == n_cores, f"need {n_cores} cores, saw {jax.devices()}"
        self.mesh = Mesh(np.asarray(devices), ("core",))
        self.pspec = PartitionSpec("core")
        in_specs = (self.pspec,) * (n_params + n_outs)
        out_specs = (self.pspec,) * len(out_names)
        self.sharded = jax.jit(
            shard_map(
                _body,
                mesh=self.mesh,
                in_specs=in_specs,
                out_specs=out_specs,
                check_rep=False,
            ),
            keep_unused=True,
        )
        self.n_params = n_params

    def prepare(self, in_maps):
        from jax.sharding import NamedSharding

        per_core = [[np.asarray(m[n]) for n in self.in_names] for m in in_maps]
        concat_in = [
            np.concatenate([per_core[c][i] for c in range(self.n_cores)], axis=0)
            for i in range(self.n_params)
        ]
        concat_zeros = [
            np.zeros((self.n_cores * z.shape[0], *z.shape[1:]), z.dtype)
            for z in self.zero_outs
        ]
        sharding = NamedSharding(self.mesh, self.pspec)
        args = [self.jax.device_put(a, sharding) for a in concat_in + concat_zeros]
        self.jax.block_until_ready(args)
        return args

    def run(self, args):
        out = self.sharded(*args)
        self.jax.block_until_ready(out)
        return out

    def results(self, out_arrs):
        return [
            {
                name: np.asarray(out_arrs[i]).reshape(
                    self.n_cores, *self.out_avals[i].shape
                )[c]
                for i, name in enumerate(self.out_names)
            }
            for c in range(self.n_cores)
        ]


_RUNNER = None


def _get_runner():
    global _RUNNER
    if _RUNNER is None:
        _RUNNER = SpmdRunner(build_nc())
    return _RUNNER


def kernel(x, w_attn, w_o):
    runner = _get_runner()
    in_maps = shard_inputs(x, w_attn, w_o)
    args = runner.prepare(in_maps)
    out = runner.run(args)
    return unshard_output(runner.results(out))

